# revision 2
# baseline (speedup 1.0000x reference)
"""Trainium2 Bass kernel for nn_AutoAttention_Layer (sparse_attention).

Math (from the reference):
    W    = softmax(mss_weight, axis=1)                      # (3,3)
    qsum = sum_j q[b,j,:]                                   # (B,D)
    ksum_s[b,d] = sum_{l < len[b]} k[b,l,s*D+d]             # (B,3,D)
    s[r,b,d]    = (sum_s W[r,s]*ksum_s[b,d]) * qsum[b,d]
    out[b,0,r*D+d] = softmax_d(s[r,b,:])
`v` is never used.

Strategy: pure data parallel over 8 NeuronCores (128 samples/core, batch on
SBUF partitions).  The masked sum over l (the only heavy op: reading all of
k) is computed as unmasked 8-row block sums (strided VectorE tensor_reduce,
one op per DMA chunk) + a masked per-block accumulate chain
(scalar_tensor_tensor with a per-partition 0/1 block mask) + a partial-block
correction using 8 rows gathered at host-computed offsets (the row *indices*
come from kes_length on the host; the row *data* is DMA'd from HBM).
"""

import numpy as np

try:
    import concourse.bacc as bacc
except ImportError:  # pragma: no cover - path fallback
    import sys

    sys.path.insert(0, "/opt/trn_rl_repo")
    import concourse.bacc as bacc

import concourse.mybir as mybir
import concourse.tile as tile
from concourse.bass_utils import run_bass_kernel_spmd

F32 = mybir.dt.float32

NCORES = 8
B = 1024
BL = B // NCORES  # 128 samples per core = SBUF partitions
LQ = 64
LK = 200
D = 64
KD = 3 * D  # 192
G = 8  # rows per l-block
NB = LK // G  # 25 blocks
LC = 40  # l rows per DMA chunk
NCH = LK // LC  # 5 chunks
BPC = LC // G  # 5 blocks per chunk

_CACHE = {}


def _build_module():
    nc = bacc.Bacc("TRN2", target_bir_lowering=False, debug=False, num_devices=NCORES)

    q_d = nc.dram_tensor("q", [BL, LQ, D], F32, kind="ExternalInput").ap()
    k_d = nc.dram_tensor("k", [BL, LK, KD], F32, kind="ExternalInput").ap()
    kg_d = nc.dram_tensor("kg", [BL, G, KD], F32, kind="ExternalInput").ap()
    w_d = nc.dram_tensor("w", [BL, 9], F32, kind="ExternalInput").ap()
    bm_d = nc.dram_tensor("bm", [BL, NB], F32, kind="ExternalInput").ap()
    sm_d = nc.dram_tensor("sm", [BL, G], F32, kind="ExternalInput").ap()
    out_d = nc.dram_tensor("out", [BL, KD], F32, kind="ExternalOutput").ap()

    mult = mybir.AluOpType.mult
    add = mybir.AluOpType.add
    AX = mybir.AxisListType.X

    with tile.TileContext(nc) as tc:
        with (
            tc.tile_pool(name="singles", bufs=1) as singles,
            tc.tile_pool(name="kpool", bufs=3) as kpool,
            tc.tile_pool(name="bpool", bufs=2) as bpool,
            tc.tile_pool(name="small", bufs=2) as small,
        ):
            w_t = singles.tile([BL, 9], F32)
            nc.sync.dma_start(out=w_t, in_=w_d)
            bm_t = singles.tile([BL, NB], F32)
            nc.sync.dma_start(out=bm_t, in_=bm_d)
            sm_t = singles.tile([BL, G], F32)
            nc.sync.dma_start(out=sm_t, in_=sm_d)
            kg_t = singles.tile([BL, G, KD], F32)
            nc.sync.dma_start(out=kg_t, in_=kg_d)
            q_t = singles.tile([BL, LQ, D], F32)
            nc.sync.dma_start(out=q_t, in_=q_d)

            # qsum[b,d] = sum_l q[b,l,d]: reduce innermost after (p,d,l) permute
            qs = singles.tile([BL, D], F32)
            nc.vector.reduce_sum(
                out=qs[:, :], in_=q_t[:, :, :].transpose([0, 2, 1]), axis=AX
            )

            accA = singles.tile([BL, KD], F32)
            accB = singles.tile([BL, KD], F32)
            accs = [accA, accB]
            cur = None
            n_ops = 0

            for c in range(NCH):
                kc = kpool.tile([BL, LC, KD], F32, tag="kc")
                nc.sync.dma_start(out=kc, in_=k_d[:, c * LC : (c + 1) * LC, :])
                pc = bpool.tile([BL, BPC, KD], F32, tag="pc")
                # (p, (j t), d) -> (p, j, d, t); reduce innermost t (8 rows)
                kv = kc[:, :, :].rearrange("p (j t) d -> p j d t", t=G)
                nc.vector.reduce_sum(out=pc[:, :, :], in_=kv, axis=AX)
                for j in range(BPC):
                    jg = c * BPC + j
                    dst = accs[n_ops % 2]
                    if cur is None:
                        nc.vector.tensor_scalar(
                            out=dst[:, :],
                            in0=pc[:, j, :],
                            scalar1=bm_t[:, jg : jg + 1],
                            scalar2=None,
                            op0=mult,
                        )
                    else:
                        nc.vector.scalar_tensor_tensor(
                            out=dst[:, :],
                            in0=pc[:, j, :],
                            scalar=bm_t[:, jg : jg + 1],
                            in1=cur[:, :],
                            op0=mult,
                            op1=add,
                        )
                    cur = dst
                    n_ops += 1

            # partial-block correction rows (gathered on host into kg)
            for t in range(G):
                dst = accs[n_ops % 2]
                nc.vector.scalar_tensor_tensor(
                    out=dst[:, :],
                    in0=kg_t[:, t, :],
                    scalar=sm_t[:, t : t + 1],
                    in1=cur[:, :],
                    op0=mult,
                    op1=add,
                )
                cur = dst
                n_ops += 1
            ksum = cur

            obuf = singles.tile([BL, KD], F32)
            for r in range(3):
                t1 = small.tile([BL, D], F32, tag="t1")
                nc.vector.tensor_scalar(
                    out=t1[:, :],
                    in0=ksum[:, 2 * D : 3 * D],
                    scalar1=w_t[:, 3 * r + 2 : 3 * r + 3],
                    scalar2=None,
                    op0=mult,
                )
                t2 = small.tile([BL, D], F32, tag="t2")
                nc.vector.scalar_tensor_tensor(
                    out=t2[:, :],
                    in0=ksum[:, D : 2 * D],
                    scalar=w_t[:, 3 * r + 1 : 3 * r + 2],
                    in1=t1[:, :],
                    op0=mult,
                    op1=add,
                )
                t3 = small.tile([BL, D], F32, tag="t3")
                nc.vector.scalar_tensor_tensor(
                    out=t3[:, :],
                    in0=ksum[:, 0:D],
                    scalar=w_t[:, 3 * r : 3 * r + 1],
                    in1=t2[:, :],
                    op0=mult,
                    op1=add,
                )
                s_r = small.tile([BL, D], F32, tag="sr")
                nc.vector.tensor_mul(out=s_r[:, :], in0=t3[:, :], in1=qs[:, :])
                mx = small.tile([BL, 1], F32, tag="mx")
                nc.vector.reduce_max(out=mx[:, :], in_=s_r[:, :], axis=AX)
                nmx = small.tile([BL, 1], F32, tag="nmx")
                nc.vector.tensor_scalar_mul(out=nmx[:, :], in0=mx[:, :], scalar1=-1.0)
                ex = small.tile([BL, D], F32, tag="ex")
                esum = small.tile([BL, 1], F32, tag="esum")
                nc.scalar.activation(
                    out=ex[:, :],
                    in_=s_r[:, :],
                    func=mybir.ActivationFunctionType.Exp,
                    bias=nmx[:, :],
                    scale=1.0,
                    accum_out=esum[:, :],
                )
                rec = small.tile([BL, 1], F32, tag="rec")
                nc.vector.reciprocal(out=rec[:, :], in_=esum[:, :])
                nc.scalar.activation(
                    out=obuf[:, r * D : (r + 1) * D],
                    in_=ex[:, :],
                    func=mybir.ActivationFunctionType.Copy,
                    bias=0.0,
                    scale=rec[:, :],
                )

            nc.sync.dma_start(out=out_d, in_=obuf[:, :])

    nc.compile()
    return nc


def _get_module():
    nc = _CACHE.get("nc")
    if nc is None:
        nc = _build_module()
        _CACHE["nc"] = nc
    return nc


def _prepare_in_maps(q, k, kes, W):
    lens = kes.reshape(B).astype(np.int64)
    j0 = lens // G
    rem = lens % G
    rows = (j0[:, None] * G + np.arange(G)[None, :]).clip(0, LK - 1)  # (B, G)
    kg = k[np.arange(B)[:, None], rows, :]  # (B, G, KD)
    bm = ((np.arange(NB)[None, :] + 1) * G <= lens[:, None]).astype(np.float32)
    sm = (np.arange(G)[None, :] < rem[:, None]).astype(np.float32)
    w_rep = np.tile(W.reshape(1, 9), (BL, 1)).astype(np.float32)

    in_maps = []
    for c in range(NCORES):
        s = slice(c * BL, (c + 1) * BL)
        in_maps.append(
            {
                "q": np.ascontiguousarray(q[s]),
                "k": np.ascontiguousarray(k[s]),
                "kg": np.ascontiguousarray(kg[s]),
                "w": w_rep,
                "bm": np.ascontiguousarray(bm[s]),
                "sm": np.ascontiguousarray(sm[s]),
            }
        )
    return in_maps


def _run(q, k, kes_length, mss_weight, **run_kwargs):
    q = np.ascontiguousarray(np.asarray(q, dtype=np.float32))
    k = np.ascontiguousarray(np.asarray(k, dtype=np.float32))
    kes = np.asarray(kes_length).astype(np.int32)
    m = np.asarray(mss_weight, dtype=np.float32)
    e = np.exp(m - m.max(axis=1, keepdims=True))
    W = (e / e.sum(axis=1, keepdims=True)).astype(np.float32)

    nc = _get_module()
    in_maps = _prepare_in_maps(q, k, kes, W)
    res = run_bass_kernel_spmd(nc, in_maps, core_ids=list(range(NCORES)), **run_kwargs)
    out = np.concatenate([res.results[c]["out"] for c in range(NCORES)], axis=0)
    return out.reshape(B, 1, KD).astype(np.float32), res


def kernel(q, k, v=None, kes_length=None, mss_weight=None, **_):
    out, _res = _run(q, k, kes_length, mss_weight)
    return out


# revision 4
# speedup vs baseline: 1.1256x; 1.1256x over previous
"""Trainium2 Bass kernel for nn_AutoAttention_Layer (sparse_attention).

Math (from the reference):
    W    = softmax(mss_weight, axis=1)                      # (3,3)
    qsum = sum_j q[b,j,:]                                   # (B,D)
    ksum_s[b,d] = sum_{l < len[b]} k[b,l,s*D+d]             # (B,3,D)
    s[r,b,d]    = (sum_s W[r,s]*ksum_s[b,d]) * qsum[b,d]
    out[b,0,r*D+d] = softmax_d(s[r,b,:])
`v` is never used.

Strategy: pure data parallel over 8 NeuronCores (128 samples/core, batch on
SBUF partitions).  The masked sum over l (the only heavy op: reading all of
k) is computed as unmasked 8-row block sums via contiguous pairwise
tensor_tensor add trees (VectorE + GpSimd split), then a masked per-block
accumulate chain (scalar_tensor_tensor with per-partition 0/1 block masks)
plus a partial-block correction from 8 rows gathered at host-computed
offsets (indices come from kes_length on the host; row data is DMA'd from
HBM).
"""

import numpy as np

try:
    import concourse.bacc as bacc
except ImportError:  # pragma: no cover - path fallback
    import sys

    sys.path.insert(0, "/opt/trn_rl_repo")
    import concourse.bacc as bacc

import concourse.mybir as mybir
import concourse.tile as tile
from concourse.bass_utils import run_bass_kernel_spmd

F32 = mybir.dt.float32

NCORES = 8
B = 1024
BL = B // NCORES  # 128 samples per core = SBUF partitions
LQ = 64
LK = 200
D = 64
KD = 3 * D  # 192
G = 8  # rows per l-block
NB = LK // G  # 25 blocks
CHUNKS = [40, 40, 40, 40, 24, 16]  # l rows per DMA chunk (tail kept small)
GPSIMD_TREE_CHUNKS = {0, 1}  # chunk trees offloaded to GpSimd

_CACHE = {}


def _build_module():
    nc = bacc.Bacc("TRN2", target_bir_lowering=False, debug=False)

    q_d = nc.dram_tensor("q", [BL, LQ, D], F32, kind="ExternalInput").ap()
    k_d = nc.dram_tensor("k", [BL, LK, KD], F32, kind="ExternalInput").ap()
    kg_d = nc.dram_tensor("kg", [BL, G, KD], F32, kind="ExternalInput").ap()
    # meta = [w(9) | bm(25) | sm(8)] per partition
    meta_d = nc.dram_tensor("meta", [BL, 9 + NB + G], F32, kind="ExternalInput").ap()
    out_d = nc.dram_tensor("out", [BL, KD], F32, kind="ExternalOutput").ap()

    mult = mybir.AluOpType.mult
    add = mybir.AluOpType.add
    AX = mybir.AxisListType.X

    with tile.TileContext(nc) as tc:
        with (
            tc.tile_pool(name="singles", bufs=1) as singles,
            tc.tile_pool(name="kpool", bufs=3) as kpool,
            tc.tile_pool(name="tpool", bufs=2) as tpool,
            tc.tile_pool(name="bpool", bufs=2) as bpool,
            tc.tile_pool(name="small", bufs=2) as small,
        ):
            # --- k chunk DMAs first: they carry 87% of the bytes ---
            kcs = []
            l0 = 0
            for ci, R in enumerate(CHUNKS):
                kc = kpool.tile([BL, R, KD], F32, tag="kc")
                nc.sync.dma_start(out=kc, in_=k_d[:, l0 : l0 + R, :])
                kcs.append((kc, R))
                l0 += R
                if ci == 0:
                    kg_t = singles.tile([BL, G, KD], F32)
                    nc.sync.dma_start(out=kg_t, in_=kg_d)
                    meta_t = singles.tile([BL, 9 + NB + G], F32)
                    nc.sync.dma_start(out=meta_t, in_=meta_d)
            q_t = singles.tile([BL, LQ, D], F32)
            nc.sync.dma_start(out=q_t, in_=q_d)

            w_t = meta_t[:, 0:9]
            bm_t = meta_t[:, 9 : 9 + NB]
            sm_t = meta_t[:, 9 + NB : 9 + NB + G]

            accA = singles.tile([BL, KD], F32)
            accB = singles.tile([BL, KD], F32)
            accs = [accA, accB]
            n_ops = 0
            cur = None

            # --- partial-block correction chain first (kg lands early) ---
            for t in range(G):
                dst = accs[n_ops % 2]
                if cur is None:
                    nc.vector.tensor_scalar(
                        out=dst[:, :],
                        in0=kg_t[:, t, :],
                        scalar1=sm_t[:, t : t + 1],
                        scalar2=None,
                        op0=mult,
                    )
                else:
                    nc.vector.scalar_tensor_tensor(
                        out=dst[:, :],
                        in0=kg_t[:, t, :],
                        scalar=sm_t[:, t : t + 1],
                        in1=cur[:, :],
                        op0=mult,
                        op1=add,
                    )
                cur = dst
                n_ops += 1

            # --- qsum on GpSimd (pairwise tree over Lq) ---
            qcur = q_t[:, :, :]
            qrows = LQ
            qlv = 0
            while qrows > 1:
                qn = qrows // 2
                qt = tpool.tile([BL, qn, D], F32, tag=f"q{qlv}")
                qp = qcur.rearrange("p (a two) d -> p a two d", two=2)
                nc.gpsimd.tensor_tensor(
                    out=qt[:, :, :], in0=qp[:, :, 0, :], in1=qp[:, :, 1, :], op=add
                )
                qcur = qt[:, :, :]
                qrows = qn
                qlv += 1
            qs = qcur  # (BL, 1, D)

            # --- per-chunk: pairwise tree to 8-row block sums, then masked chain ---
            jg = 0
            for ci, (kc, R) in enumerate(kcs):
                eng = nc.gpsimd if ci in GPSIMD_TREE_CHUNKS else nc.vector
                rows = R
                curk = kc[:, :, :]
                lv = 0
                while rows > R // G:
                    nxt = rows // 2
                    t = tpool.tile([BL, nxt, KD], F32, tag=f"t{lv}")
                    pairs = curk.rearrange("p (a two) d -> p a two d", two=2)
                    eng.tensor_tensor(
                        out=t[:, :, :],
                        in0=pairs[:, :, 0, :],
                        in1=pairs[:, :, 1, :],
                        op=add,
                    )
                    curk = t[:, :, :]
                    rows = nxt
                    lv += 1
                # curk: (BL, R//G, KD) block sums
                for j in range(R // G):
                    dst = accs[n_ops % 2]
                    nc.vector.scalar_tensor_tensor(
                        out=dst[:, :],
                        in0=curk[:, j, :],
                        scalar=bm_t[:, jg : jg + 1],
                        in1=cur[:, :],
                        op0=mult,
                        op1=add,
                    )
                    cur = dst
                    n_ops += 1
                    jg += 1
            ksum = cur

            # --- mix (3x3 softmaxed weights), scale by qsum, softmax over D ---
            obuf = singles.tile([BL, KD], F32)
            for r in range(3):
                t1 = small.tile([BL, D], F32, tag="t1")
                nc.vector.tensor_scalar(
                    out=t1[:, :],
                    in0=ksum[:, 2 * D : 3 * D],
                    scalar1=w_t[:, 3 * r + 2 : 3 * r + 3],
                    scalar2=None,
                    op0=mult,
                )
                t2 = small.tile([BL, D], F32, tag="t2")
                nc.vector.scalar_tensor_tensor(
                    out=t2[:, :],
                    in0=ksum[:, D : 2 * D],
                    scalar=w_t[:, 3 * r + 1 : 3 * r + 2],
                    in1=t1[:, :],
                    op0=mult,
                    op1=add,
                )
                t3 = small.tile([BL, D], F32, tag="t3")
                nc.vector.scalar_tensor_tensor(
                    out=t3[:, :],
                    in0=ksum[:, 0:D],
                    scalar=w_t[:, 3 * r : 3 * r + 1],
                    in1=t2[:, :],
                    op0=mult,
                    op1=add,
                )
                s_r = small.tile([BL, D], F32, tag="sr")
                nc.vector.tensor_mul(out=s_r[:, :], in0=t3[:, :], in1=qs[:, 0, :])
                mx = small.tile([BL, 1], F32, tag="mx")
                nc.vector.reduce_max(out=mx[:, :], in_=s_r[:, :], axis=AX)
                nmx = small.tile([BL, 1], F32, tag="nmx")
                nc.vector.tensor_scalar_mul(out=nmx[:, :], in0=mx[:, :], scalar1=-1.0)
                ex = small.tile([BL, D], F32, tag="ex")
                esum = small.tile([BL, 1], F32, tag="esum")
                nc.scalar.activation(
                    out=ex[:, :],
                    in_=s_r[:, :],
                    func=mybir.ActivationFunctionType.Exp,
                    bias=nmx[:, :],
                    scale=1.0,
                    accum_out=esum[:, :],
                )
                rec = small.tile([BL, 1], F32, tag="rec")
                nc.vector.reciprocal(out=rec[:, :], in_=esum[:, :])
                nc.scalar.activation(
                    out=obuf[:, r * D : (r + 1) * D],
                    in_=ex[:, :],
                    func=mybir.ActivationFunctionType.Copy,
                    bias=0.0,
                    scale=rec[:, :],
                )

            nc.sync.dma_start(out=out_d, in_=obuf[:, :])

    nc.compile()
    return nc


def _get_module():
    nc = _CACHE.get("nc")
    if nc is None:
        nc = _build_module()
        _CACHE["nc"] = nc
    return nc


def _prepare_in_maps(q, k, kes, W):
    lens = kes.reshape(B).astype(np.int64)
    j0 = lens // G
    rem = lens % G
    rows = (j0[:, None] * G + np.arange(G)[None, :]).clip(0, LK - 1)  # (B, G)
    kg = k[np.arange(B)[:, None], rows, :]  # (B, G, KD)
    bm = ((np.arange(NB)[None, :] + 1) * G <= lens[:, None]).astype(np.float32)
    sm = (np.arange(G)[None, :] < rem[:, None]).astype(np.float32)
    w_rep = np.tile(W.reshape(1, 9), (B, 1)).astype(np.float32)
    meta = np.concatenate([w_rep, bm, sm], axis=1).astype(np.float32)  # (B, 42)

    in_maps = []
    for c in range(NCORES):
        s = slice(c * BL, (c + 1) * BL)
        in_maps.append(
            {
                "q": np.ascontiguousarray(q[s]),
                "k": np.ascontiguousarray(k[s]),
                "kg": np.ascontiguousarray(kg[s]),
                "meta": np.ascontiguousarray(meta[s]),
            }
        )
    return in_maps


def _run(q, k, kes_length, mss_weight, **run_kwargs):
    q = np.ascontiguousarray(np.asarray(q, dtype=np.float32))
    k = np.ascontiguousarray(np.asarray(k, dtype=np.float32))
    kes = np.asarray(kes_length).astype(np.int32)
    m = np.asarray(mss_weight, dtype=np.float32)
    e = np.exp(m - m.max(axis=1, keepdims=True))
    W = (e / e.sum(axis=1, keepdims=True)).astype(np.float32)

    nc = _get_module()
    in_maps = _prepare_in_maps(q, k, kes, W)
    res = run_bass_kernel_spmd(nc, in_maps, core_ids=list(range(NCORES)), **run_kwargs)
    out = np.concatenate([res.results[c]["out"] for c in range(NCORES)], axis=0)
    return out.reshape(B, 1, KD).astype(np.float32), res


def kernel(q, k, v=None, kes_length=None, mss_weight=None, **_):
    out, _res = _run(q, k, kes_length, mss_weight)
    return out
